# revision 21
# baseline (speedup 1.0000x reference)
"""Trainium2 Bass kernel for a 4-step differentiable recurrent net forward pass.

Reference computation (B=8192, NI=512, NH=2048, NO=512, 4 steps):
    activs = 0; outputs = 0
    repeat 4x:  pre = hr * (x @ Wih.T + activs @ Whh.T + outputs @ Woh.T) + hb
                activs = per_neuron_act(pre)        # tanh/sigmoid/relu by i%3
    out = sigmoid(or * (x @ Wio.T + outputs @ Woo.T + activs @ Who.T) + ob)

`outputs` is never written inside the loop, so the Woh/Woo terms vanish and
the x-projection P = hr*(x@Wih.T)+hb is loop-invariant (computed once).

Strategy: data-parallel on batch across 8 cores (1024 rows each). On-core
everything is feature-major (features on SBUF partitions, batch on the free
axis), so each matmul is W_tile.T @ X^T with stationary weights. The
recurrent Whh and the output Who matmuls (91% of tensor work) run in
fp8 e4m3 with perf_mode=DoubleRow: K=256 contraction per instruction at 2
MACs/PE-cycle. DoubleRow LDWEIGHTS (256 columns, no FWL) costs more than
the matmul itself, so both 512-row batch chunks are computed back-to-back
under one weight load: bass emits an InstLdweights per matmul, and a
post-build pass (_dedupe_ldweights) removes the redundant second load so
the non-self-loading second matmul reuses the array-resident weights.
Weights are scaled x32 before the fp8 cast (keeps them out of the
subnormal range); every PSUM eviction goes through the ACT engine with
scale=1/32 to compensate. Eviction temporaries are fp16 (the ACT engine
reads 2B/cycle/lane, so f32 sources run at half rate) and activations are
written as fp8 directly by the ACT engine. The input projections (x@Wih,
x@Wio) are fp8 DoubleRow as well — simulated end-to-end rel err 1.27e-2
vs the 2e-2 budget. Host-side prep: hidden neurons are permuted
so the three activation groups are contiguous, hr/or are folded into the
weight matrices, weights are packed so each loads as one large contiguous
DMA, and hb/ob are applied as per-partition bias APs.
"""

import os

import numpy as np
import ml_dtypes

import concourse.bass as bass
import concourse.tile as tile
from concourse import bacc, mybir
from concourse.bass_utils import run_bass_kernel_spmd

B, NI, NH, NO = 8192, 512, 2048, 512
N_STEPS = 4
N_CORES = 8
BL = B // N_CORES          # batch rows per core
CH = 512                   # batch chunk (max moving free dim)
NCH = BL // CH             # 2 chunks per core
KI = NI // 128             # 4 k-tiles over inputs
KH = NH // 128             # 16 k/m-tiles over hidden
KP = KH // 2               # 8 DoubleRow k-pairs over hidden
KO = NO // 128             # 4 m-tiles over outputs

BF16 = mybir.dt.bfloat16
F16 = mybir.dt.float16
F32 = mybir.dt.float32
FP8 = mybir.dt.float8e4
AF = mybir.ActivationFunctionType
DR = mybir.MatmulPerfMode.DoubleRow

SCALE = 32.0               # fp8 weight pre-scale; undone by ACT scale=1/SCALE
INV = 1.0 / SCALE

# hidden neurons regrouped as [all tanh | all sigmoid | all relu]
_idx = np.arange(NH)
PERM = np.concatenate([_idx[_idx % 3 == 0], _idx[_idx % 3 == 1], _idx[_idx % 3 == 2]])
_B1 = int((_idx % 3 == 0).sum())           # 683
_B2 = _B1 + int((_idx % 3 == 1).sum())     # 1366

# per m-tile: the single activation function, or None for the two mixed tiles
_TILE_FUNC = []
for _m in range(KH):
    _lo, _hi = _m * 128, (_m + 1) * 128
    _fs = set()
    for _f, _a, _b in ((AF.Tanh, 0, _B1), (AF.Sigmoid, _B1, _B2), (AF.Relu, _B2, NH)):
        if max(_lo, _a) < min(_hi, _b):
            _fs.add(_f)
    _TILE_FUNC.append(_fs.pop() if len(_fs) == 1 else None)

# mixed tiles: (major_func applied everywhere, minor_func, mask column block)
# partition sub-ranges must be 32-aligned on TRN2, so the minority strip is
# fixed up with a full-tile ACT + copy_predicated against a {0,1} mask
_BOUNDARY = {
    _B1 // 128: (AF.Sigmoid, AF.Tanh, 0),    # tile 5: parts < 43 are tanh
    _B2 // 128: (AF.Sigmoid, AF.Relu, 1),    # tile 10: parts >= 86 are relu
}


def _dedupe_ldweights(nc):
    """Drop an InstLdweights that reloads exactly what the PE already holds.

    bass splits every matmul into InstLdweights + non-self-loading
    InstMatmult at build time; consecutive matmuls on the same stationary
    tile therefore carry a redundant (and expensive, for DoubleRow) reload.
    Only sync-free duplicates are removed, and any non-matmul PE
    instruction invalidates the tracked weights.
    """
    removed = 0
    for blk in nc.main_func.blocks:
        prev_key = None
        to_remove = []
        for i in blk.instructions:
            tn = type(i).__name__
            if tn == "InstLdweights":
                k = (repr(i.ins[0]), repr(i.perf_mode), repr(i.is_transpose),
                     repr(i.tile_position), repr(i.tile_size))
                si = i.sync_info
                clean = si is None or (len(si.on_wait) == 0
                                       and len(si.on_update) == 0)
                if k == prev_key and clean:
                    to_remove.append(i)
                else:
                    prev_key = k
            elif tn == "InstMatmult":
                continue
            elif getattr(i, "engine", None) == mybir.EngineType.PE:
                prev_key = None
        for i in to_remove:
            blk.instructions.remove(i)
        removed += len(to_remove)
    return removed


def _emit_hidden_act2(nc, ps, blk2, a_new, tmp_pool, bmask_t):
    """Evict a 2-m-tile pre-activation slab through the grouped activations.

    ps:    SBUF AP (128, 2*CH) fp16/bf16 holding SCALE*pre for m-tiles
           2*blk2, 2*blk2+1
    a_new: SBUF tile (128, KH, CH) fp8, m-tile m lives at [:, m, :]
    """
    mloc = 0
    while mloc < 2:
        m = 2 * blk2 + mloc
        if m in _BOUNDARY:
            major, minor, mb = _BOUNDARY[m]
            nc.scalar.activation(
                a_new[:, m, :], ps[:, mloc * CH:(mloc + 1) * CH], major,
                scale=INV)
            t = tmp_pool.tile([128, CH], FP8, tag="btmp", bufs=2, name="btmp")
            nc.scalar.activation(t[:], ps[:, mloc * CH:(mloc + 1) * CH], minor,
                                 scale=INV)
            nc.vector.copy_predicated(
                a_new[:, m, :], bmask_t[:, mb * CH:(mb + 1) * CH], t[:])
            mloc += 1
            continue
        func = _TILE_FUNC[m]
        end = mloc + 1
        while end < 2 and _TILE_FUNC[2 * blk2 + end] == func:
            end += 1
        nc.scalar.activation(
            a_new[:, 2 * blk2 + mloc:2 * blk2 + end, :],
            ps[:, mloc * CH:end * CH], func, scale=INV)
        mloc = end


def _build_nc():
    nc = bacc.Bacc("TRN2", target_bir_lowering=False, debug=False,
                   num_devices=N_CORES, dynamic_dma_scratch_size=2048)

    # all operands fp8, host-packed for DoubleRow APs with DMA arrival order
    # matching compute order:
    # x:   [p, (c*2+t)*2+i, col] = x.T[(2t+i)*128+p, c*CH+col]
    # wih: m-block-major [p, ((b*2+t)*2+mloc)*2+i, j]
    #        = Wih_s[(2t+i)*128+p, (2b+mloc)*128+j]
    # wio: [p, (t*4+mo)*2+i, j] = Wio_s[(2t+i)*128+p, mo*128+j]
    # whh: m-block-major [p, ((b*8+t)*2+mloc)*2+i, j]
    #        = Whh_s[(2t+i)*128+p, (2b+mloc)*128+j]
    # who: [p, (t*4+mo)*2+i, j] = Who_s[(2t+i)*128+p, mo*128+j]
    xT = nc.dram_tensor("xT", [128, NCH * 4, CH], FP8,
                        kind="ExternalInput").ap()
    wih = nc.dram_tensor("wih", [128, 8 * 4 * 2, 128], FP8,
                         kind="ExternalInput").ap()
    whh = nc.dram_tensor("whh", [128, KP * KH * 2, 128], FP8,
                         kind="ExternalInput").ap()
    who = nc.dram_tensor("who", [128, KP * KO * 2, 128], FP8,
                         kind="ExternalInput").ap()
    wio = nc.dram_tensor("wio", [128, 2 * KO * 2, 128], FP8,
                         kind="ExternalInput").ap()
    hbc = nc.dram_tensor("hbc", [128, KH], F32, kind="ExternalInput").ap()
    obc = nc.dram_tensor("obc", [128, KO], F32, kind="ExternalInput").ap()
    bmask = nc.dram_tensor("bmask", [128, 2 * CH], mybir.dt.uint8,
                           kind="ExternalInput").ap()
    outT = nc.dram_tensor("outT", [NO, BL], BF16, kind="ExternalOutput").ap()

    with tile.TileContext(nc) as tc:
        with tc.tile_pool(name="w", bufs=1) as wpool, \
             tc.tile_pool(name="act", bufs=1) as apool, \
             tc.tile_pool(name="ps", bufs=2, space="PSUM") as pspool, \
             tc.tile_pool(name="out", bufs=4) as opool:

            # ---- stage inputs. wih lands as one 128KB DMA per 2-m-tile
            # block (sync queue) in the exact order the P phase consumes
            # them; x lands chunk-major on the scalar queue; the 4MB whh
            # follows split across both queues.
            wih_m = wpool.tile([128, 8 * 4 * 2, 128], FP8, tag="wih",
                               name="wihm")
            x_m = wpool.tile([128, NCH * 4, CH], FP8, tag="x", name="xm")
            nc.sync.dma_start(wih_m[:, 0:8, :], wih[:, 0:8, :])
            nc.scalar.dma_start(x_m[:, 0:4, :], xT[:, 0:4, :])
            for b in range(1, 8):
                nc.sync.dma_start(wih_m[:, b * 8:(b + 1) * 8, :],
                                  wih[:, b * 8:(b + 1) * 8, :])
            nc.scalar.dma_start(x_m[:, 4:8, :], xT[:, 4:8, :])
            hbc_t = wpool.tile([128, KH], F32, tag="hbc")
            nc.scalar.dma_start(hbc_t[:], hbc[:])
            obc_t = wpool.tile([128, KO], F32, tag="obc")
            nc.scalar.dma_start(obc_t[:], obc[:])
            bmask_t = wpool.tile([128, 2 * CH], mybir.dt.uint8, tag="bmask")
            nc.scalar.dma_start(bmask_t[:], bmask[:])
            wio_m = wpool.tile([128, 2 * KO * 2, 128], FP8, tag="wio",
                               name="wiom")
            nc.scalar.dma_start(wio_m[:], wio[:])
            # whh fp8: 4MB total as 8x 0.5MB DMAs over sync+scalar queues,
            # landing in the order hh step 1 consumes its 2-m-tile blocks
            whh_m = wpool.tile([128, KP * KH * 2, 128], FP8, tag="whh",
                               name="whhm")
            for b in range(8):
                eng = nc.sync if b % 2 == 0 else nc.scalar
                eng.dma_start(whh_m[:, b * 32:(b + 1) * 32, :],
                              whh[:, b * 32:(b + 1) * 32, :])
            who_m = wpool.tile([128, KP * KO * 2, 128], FP8, tag="who",
                               name="whom")
            nc.scalar.dma_start(who_m[:], who[:])

            def x8_ap(t, c):
                lo = (c * 2 + t) * 2
                return x_m[:, lo:lo + 2, :]

            def wih8_ap(t, m):
                lo = ((m // 2 * 2 + t) * 2 + m % 2) * 2
                return wih_m[:, lo:lo + 2, :]

            def wio8_ap(t, mo):
                lo = (t * KO + mo) * 2
                return wio_m[:, lo:lo + 2, :]

            def psum2(i):
                # two 2-bank accumulators live at once (one per chunk, or
                # pipelined across 2-m-tile blocks); bufs=2 each fills PSUM
                return pspool.tile([128, 2 * CH], F32,
                                   tag=("psA" if i % 2 == 0 else "psB"),
                                   bufs=2, name="psb")

            # ---- x-projection P (= SCALE*(hr*(x@Wih.T)+hb), bf16) and
            # first-step activations: fp8 DoubleRow (K=512 = 2 pairs), both
            # chunks per weight load, 2-m-tile blocks ----
            P = {}
            A = {}
            for c in range(NCH):
                P[c] = apool.tile([128, KH * CH], BF16, tag=f"P{c}",
                                  name=f"P{c}")
                A[c] = apool.tile([128, KH, CH], FP8, tag=f"A{c}", bufs=2,
                                  name=f"A1c{c}")
            for blk in range(8):
                pss = [psum2(0), psum2(1)]
                for t in range(2):
                    for mloc in range(2):
                        m = 2 * blk + mloc
                        for c in range(NCH):
                            nc.tensor.matmul(
                                pss[c][:, mloc * CH:(mloc + 1) * CH],
                                wih8_ap(t, m), x8_ap(t, c),
                                start=(t == 0), stop=(t == 1), perf_mode=DR)
                for c in range(NCH):
                    for mloc in range(2):
                        m = 2 * blk + mloc
                        nc.vector.tensor_scalar_add(
                            P[c][:, m * CH:(m + 1) * CH],
                            pss[c][:, mloc * CH:(mloc + 1) * CH],
                            hbc_t[:, m:m + 1])
                    # A1 = act(P/SCALE) straight from SBUF — frees the PSUM
                    # slot as soon as the adds have read it
                    _emit_hidden_act2(
                        nc, P[c][:, 2 * blk * CH:(2 * blk + 2) * CH],
                        blk, A[c], opool, bmask_t)

            # ---- whh-independent output x-projection (fills the window
            # while the 4MB whh load is still in flight); holds SCALE*or*
            # (x@Wio.T) ----
            outx = {}
            for c in range(NCH):
                outx[c] = apool.tile([128, KO * CH], BF16, tag=f"outx{c}",
                                     name=f"outx{c}")
            for half in range(2):
                pss = [psum2(0), psum2(1)]
                for t in range(2):
                    for mloc in range(2):
                        mo = 2 * half + mloc
                        for c in range(NCH):
                            nc.tensor.matmul(
                                pss[c][:, mloc * CH:(mloc + 1) * CH],
                                wio8_ap(t, mo), x8_ap(t, c),
                                start=(t == 0), stop=(t == 1), perf_mode=DR)
                for c in range(NCH):
                    nc.vector.tensor_copy(
                        outx[c][:, 2 * half * CH:(2 * half + 2) * CH],
                        pss[c][:])

            # ---- recurrent steps 2..4: fp8 DoubleRow, both chunks computed
            # back-to-back per weight load (the dedupe pass removes the
            # second LDWEIGHTS) ----
            def hh_step_fused(s):
                a_new = [apool.tile([128, KH, CH], FP8, tag=f"A{c}", bufs=2,
                                    name=f"A{s + 2}c{c}") for c in range(NCH)]
                for blk in range(8):
                    pss = [psum2(0), psum2(1)]
                    for t in range(KP):
                        for mloc in range(2):
                            m = 2 * blk + mloc
                            w2 = ((blk * KP + t) * 2 + mloc) * 2
                            for c in range(NCH):
                                nc.tensor.matmul(
                                    pss[c][:, mloc * CH:(mloc + 1) * CH],
                                    whh_m[:, w2:w2 + 2, :],
                                    A[c][:, 2 * t:2 * t + 2, :],
                                    start=(t == 0), stop=(t == KP - 1),
                                    perf_mode=DR)
                    for c in range(NCH):
                        # pre = psum + P into an fp16 SBUF temp: a single
                        # PSUM read frees the bank, and the ACT engine reads
                        # 16-bit sources at full rate
                        tmp = opool.tile([128, 2 * CH], F16, tag="pre",
                                         bufs=4, name="pre")
                        nc.vector.tensor_add(
                            tmp[:], pss[c][:],
                            P[c][:, 2 * blk * CH:(2 * blk + 2) * CH])
                        _emit_hidden_act2(nc, tmp, blk, a_new[c], opool,
                                          bmask_t)
                for c in range(NCH):
                    A[c] = a_new[c]

            for s in range(N_STEPS - 1):
                hh_step_fused(s)

            # ---- output layer (fp8 DoubleRow, same weight reuse) ----
            for mo in range(KO):
                pss = [psum2(0), psum2(1)]
                for t in range(KP):
                    w2 = (t * KO + mo) * 2
                    for c in range(NCH):
                        nc.tensor.matmul(
                            pss[c][:, 0:CH],
                            who_m[:, w2:w2 + 2, :],
                            A[c][:, 2 * t:2 * t + 2, :],
                            start=(t == 0), stop=(t == KP - 1),
                            perf_mode=DR)
                # half-tile evictions keep the post-matmul drain short
                for c in range(NCH):
                    for h in range(2):
                        lo, hi = h * (CH // 2), (h + 1) * (CH // 2)
                        to = opool.tile([128, CH // 2], F16, tag="preo",
                                        bufs=4, name="preo")
                        nc.vector.tensor_add(
                            to[:], pss[c][:, lo:hi],
                            outx[c][:, mo * CH + lo:mo * CH + hi])
                        o = opool.tile([128, CH // 2], BF16, tag="o", bufs=4,
                                       name="o")
                        nc.scalar.activation(o[:], to[:], AF.Sigmoid,
                                             bias=obc_t[:, mo:mo + 1],
                                             scale=INV)
                        eng = nc.sync if (c + h) % 2 == 0 else nc.scalar
                        eng.dma_start(
                            outT[mo * 128:(mo + 1) * 128,
                                 c * CH + lo:c * CH + hi],
                            o[:])

    _dedupe_ldweights(nc)
    nc.compile()
    return nc


_NC_CACHE = None


def _get_nc():
    global _NC_CACHE
    if _NC_CACHE is None:
        _NC_CACHE = _build_nc()
    return _NC_CACHE


def _make_bmask():
    m = np.zeros((128, 2 * CH), np.uint8)
    m[:_B1 - (_B1 // 128) * 128, 0:CH] = 1          # tile 5: parts < 43 tanh
    m[_B2 - (_B2 // 128) * 128:, CH:2 * CH] = 1     # tile 10: parts >= 86 relu
    return m


def _pack_dr(w_s, kp, mt):
    """(kp*256, mt*128) k-major weights -> DoubleRow layout
    [128, kp*mt*2, 128]: [p, (t*mt+m)*2+i, j] = w_s[(2t+i)*128+p, m*128+j]
    """
    fp8 = ml_dtypes.float8_e4m3
    w4 = w_s.reshape(kp, 2, 128, mt, 128)            # t, i, p, m, j
    return np.ascontiguousarray(
        w4.transpose(2, 0, 3, 1, 4).reshape(128, kp * mt * 2, 128)).astype(fp8)


def _prep_in_maps(inputs):
    bf = ml_dtypes.bfloat16
    x = np.asarray(inputs["inputs"], np.float32)
    hr = np.asarray(inputs["hidden_responses"], np.float32)[PERM]
    hb = np.asarray(inputs["hidden_biases"], np.float32)[PERM]
    orr = np.asarray(inputs["output_responses"], np.float32)
    ob = np.asarray(inputs["output_biases"], np.float32)

    wih_s = SCALE * (hr[:, None] *
                     np.asarray(inputs["input_to_hidden"], np.float32)[PERM]).T
    whh_s = SCALE * (hr[:, None] *
                     np.asarray(inputs["hidden_to_hidden"],
                                np.float32)[PERM][:, PERM]).T
    who_s = SCALE * (orr[:, None] *
                     np.asarray(inputs["hidden_to_output"],
                                np.float32)[:, PERM]).T
    wio_s = SCALE * (orr[:, None] *
                     np.asarray(inputs["input_to_output"], np.float32)).T

    fp8 = ml_dtypes.float8_e4m3
    # wih m-block-major DR: [p, ((b*2+t)*2+mloc)*2+i, j]
    #   = wih_s[(2t+i)*128+p, (2b+mloc)*128+j]
    wih_p = np.ascontiguousarray(
        wih_s.reshape(2, 2, 128, 8, 2, 128).transpose(2, 3, 0, 4, 1, 5)
        .reshape(128, 64, 128)).astype(fp8)

    shared = {
        "wih": wih_p,
        "whh": np.ascontiguousarray(
            whh_s.reshape(KP, 2, 128, 8, 2, 128).transpose(2, 3, 0, 4, 1, 5)
            .reshape(128, KP * KH * 2, 128)).astype(fp8),
        "who": _pack_dr(who_s, KP, KO),
        "wio": _pack_dr(wio_s, 2, KO),
        "hbc": np.ascontiguousarray(SCALE * hb.reshape(KH, 128).T),
        "obc": np.ascontiguousarray(ob.reshape(KO, 128).T),
        "bmask": _make_bmask(),
    }
    in_maps = []
    for c in range(N_CORES):
        m = dict(shared)
        # x chunk-major DR: [p, (ch*2+t)*2+i, col]
        #   = x.T[(2t+i)*128+p, ch*CH+col]
        xc = np.ascontiguousarray(x[c * BL:(c + 1) * BL].T)     # (NI, BL)
        m["xT"] = np.ascontiguousarray(
            xc.reshape(2, 2, 128, NCH, CH).transpose(2, 3, 0, 1, 4)
            .reshape(128, NCH * 4, CH)).astype(fp8)
        in_maps.append(m)
    return in_maps


def _run(inputs, trace=False, tmpdir=None):
    nc = _get_nc()
    in_maps = _prep_in_maps(inputs)
    res = run_bass_kernel_spmd(nc, in_maps, core_ids=list(range(N_CORES)),
                               trace=trace, tmpdir=tmpdir)
    out = np.empty((B, NO), np.float32)
    for c in range(N_CORES):
        out[c * BL:(c + 1) * BL] = res.results[c]["outT"].T.astype(np.float32)
    return out, res


def kernel(**inputs) -> np.ndarray:
    out, _ = _run(inputs, trace=False)
    return out


if __name__ == "__main__":
    rng = np.random.default_rng(0)
    ins = {
        "inputs": rng.standard_normal((B, NI), dtype=np.float32),
        "input_to_hidden": rng.standard_normal((NH, NI), dtype=np.float32) * 0.02,
        "hidden_to_hidden": rng.standard_normal((NH, NH), dtype=np.float32) * 0.02,
        "output_to_hidden": rng.standard_normal((NH, NO), dtype=np.float32) * 0.02,
        "input_to_output": rng.standard_normal((NO, NI), dtype=np.float32) * 0.02,
        "hidden_to_output": rng.standard_normal((NO, NH), dtype=np.float32) * 0.02,
        "output_to_output": rng.standard_normal((NO, NO), dtype=np.float32) * 0.02,
        "hidden_responses": rng.standard_normal(NH, dtype=np.float32) * 0.1 + 1.0,
        "hidden_biases": rng.standard_normal(NH, dtype=np.float32) * 0.1,
        "output_responses": rng.standard_normal(NO, dtype=np.float32) * 0.1 + 1.0,
        "output_biases": rng.standard_normal(NO, dtype=np.float32) * 0.1,
    }
    out = kernel(**ins)
    print("kernel output", out.shape, out.dtype, out[:2, :4])


# revision 26
# speedup vs baseline: 1.0042x; 1.0042x over previous
"""Trainium2 Bass kernel for a 4-step differentiable recurrent net forward pass.

Reference computation (B=8192, NI=512, NH=2048, NO=512, 4 steps):
    activs = 0; outputs = 0
    repeat 4x:  pre = hr * (x @ Wih.T + activs @ Whh.T + outputs @ Woh.T) + hb
                activs = per_neuron_act(pre)        # tanh/sigmoid/relu by i%3
    out = sigmoid(or * (x @ Wio.T + outputs @ Woo.T + activs @ Who.T) + ob)

`outputs` is never written inside the loop, so the Woh/Woo terms vanish and
the x-projection P = hr*(x@Wih.T)+hb is loop-invariant (computed once).

Strategy: data-parallel on batch across 8 cores (1024 rows each). On-core
everything is feature-major (features on SBUF partitions, batch on the free
axis), so each matmul is W_tile.T @ X^T with stationary weights. The
recurrent Whh and the output Who matmuls (91% of tensor work) run in
fp8 e4m3 with perf_mode=DoubleRow: K=256 contraction per instruction at 2
MACs/PE-cycle. DoubleRow LDWEIGHTS (256 columns, no FWL) costs more than
the matmul itself, so both 512-row batch chunks are computed back-to-back
under one weight load: bass emits an InstLdweights per matmul, and a
post-build pass (_dedupe_ldweights) removes the redundant second load so
the non-self-loading second matmul reuses the array-resident weights.
Weights are scaled x32 before the fp8 cast (keeps them out of the
subnormal range); every PSUM eviction goes through the ACT engine with
scale=1/32 to compensate. Eviction temporaries are fp16 (the ACT engine
reads 2B/cycle/lane, so f32 sources run at half rate) and activations are
written as fp8 directly by the ACT engine. The input projections (x@Wih,
x@Wio) are fp8 DoubleRow as well — simulated end-to-end rel err 1.27e-2
vs the 2e-2 budget. Host-side prep: hidden neurons are permuted
so the three activation groups are contiguous, hr/or are folded into the
weight matrices, weights are packed so each loads as one large contiguous
DMA, and hb/ob are applied as per-partition bias APs.
"""

import os

import numpy as np
import ml_dtypes

import concourse.bass as bass
import concourse.tile as tile
from concourse import bacc, mybir
from concourse.bass_utils import run_bass_kernel_spmd

B, NI, NH, NO = 8192, 512, 2048, 512
N_STEPS = 4
N_CORES = 8
BL = B // N_CORES          # batch rows per core
CH = 512                   # batch chunk (max moving free dim)
NCH = BL // CH             # 2 chunks per core
KI = NI // 128             # 4 k-tiles over inputs
KH = NH // 128             # 16 k/m-tiles over hidden
KP = KH // 2               # 8 DoubleRow k-pairs over hidden
KO = NO // 128             # 4 m-tiles over outputs

BF16 = mybir.dt.bfloat16
F16 = mybir.dt.float16
F32 = mybir.dt.float32
FP8 = mybir.dt.float8e4
AF = mybir.ActivationFunctionType
DR = mybir.MatmulPerfMode.DoubleRow

SCALE = 32.0               # fp8 weight pre-scale; undone by ACT scale=1/SCALE
INV = 1.0 / SCALE

# hidden neurons regrouped as [all tanh | all sigmoid | all relu]
_idx = np.arange(NH)
PERM = np.concatenate([_idx[_idx % 3 == 0], _idx[_idx % 3 == 1], _idx[_idx % 3 == 2]])
_B1 = int((_idx % 3 == 0).sum())           # 683
_B2 = _B1 + int((_idx % 3 == 1).sum())     # 1366

# per m-tile: the single activation function, or None for the two mixed tiles
_TILE_FUNC = []
for _m in range(KH):
    _lo, _hi = _m * 128, (_m + 1) * 128
    _fs = set()
    for _f, _a, _b in ((AF.Tanh, 0, _B1), (AF.Sigmoid, _B1, _B2), (AF.Relu, _B2, NH)):
        if max(_lo, _a) < min(_hi, _b):
            _fs.add(_f)
    _TILE_FUNC.append(_fs.pop() if len(_fs) == 1 else None)

# mixed tiles: (major_func applied everywhere, minor_func, mask column block)
# partition sub-ranges must be 32-aligned on TRN2, so the minority strip is
# fixed up with a full-tile ACT + copy_predicated against a {0,1} mask
_BOUNDARY = {
    _B1 // 128: (AF.Sigmoid, AF.Tanh, 0),    # tile 5: parts < 43 are tanh
    _B2 // 128: (AF.Sigmoid, AF.Relu, 1),    # tile 10: parts >= 86 are relu
}


def _dedupe_ldweights(nc):
    """Drop an InstLdweights that reloads exactly what the PE already holds.

    bass splits every matmul into InstLdweights + non-self-loading
    InstMatmult at build time; consecutive matmuls on the same stationary
    tile therefore carry a redundant (and expensive, for DoubleRow) reload.
    Only sync-free duplicates are removed, and any non-matmul PE
    instruction invalidates the tracked weights.
    """
    removed = 0
    for blk in nc.main_func.blocks:
        prev_key = None
        to_remove = []
        for i in blk.instructions:
            tn = type(i).__name__
            if tn == "InstLdweights":
                k = (repr(i.ins[0]), repr(i.perf_mode), repr(i.is_transpose),
                     repr(i.tile_position), repr(i.tile_size))
                si = i.sync_info
                clean = si is None or (len(si.on_wait) == 0
                                       and len(si.on_update) == 0)
                if k == prev_key and clean:
                    to_remove.append(i)
                else:
                    prev_key = k
            elif tn == "InstMatmult":
                continue
            elif getattr(i, "engine", None) == mybir.EngineType.PE:
                prev_key = None
        for i in to_remove:
            blk.instructions.remove(i)
        removed += len(to_remove)
    return removed


def _emit_hidden_act2(nc, ps, blk2, a_new, tmp_pool, bmask_t):
    """Evict a 2-m-tile pre-activation slab through the grouped activations.

    ps:    SBUF AP (128, 2*CH) fp16/bf16 holding SCALE*pre for m-tiles
           2*blk2, 2*blk2+1
    a_new: SBUF tile (128, KH, CH) fp8, m-tile m lives at [:, m, :]
    """
    mloc = 0
    while mloc < 2:
        m = 2 * blk2 + mloc
        if m in _BOUNDARY:
            major, minor, mb = _BOUNDARY[m]
            nc.scalar.activation(
                a_new[:, m, :], ps[:, mloc * CH:(mloc + 1) * CH], major,
                scale=INV)
            t = tmp_pool.tile([128, CH], FP8, tag="btmp", bufs=2, name="btmp")
            nc.scalar.activation(t[:], ps[:, mloc * CH:(mloc + 1) * CH], minor,
                                 scale=INV)
            nc.vector.copy_predicated(
                a_new[:, m, :], bmask_t[:, mb * CH:(mb + 1) * CH], t[:])
            mloc += 1
            continue
        func = _TILE_FUNC[m]
        end = mloc + 1
        while end < 2 and _TILE_FUNC[2 * blk2 + end] == func:
            end += 1
        nc.scalar.activation(
            a_new[:, 2 * blk2 + mloc:2 * blk2 + end, :],
            ps[:, mloc * CH:end * CH], func, scale=INV)
        mloc = end


def _build_nc():
    nc = bacc.Bacc("TRN2", target_bir_lowering=False, debug=False,
                   num_devices=N_CORES, dynamic_dma_scratch_size=2048)

    # all operands fp8, host-packed for DoubleRow APs with DMA arrival order
    # matching compute order:
    # x:   [p, (c*2+t)*2+i, col] = x.T[(2t+i)*128+p, c*CH+col]
    # wih: m-block-major [p, ((b*2+t)*2+mloc)*2+i, j]
    #        = Wih_s[(2t+i)*128+p, (2b+mloc)*128+j]
    # wio: [p, (t*4+mo)*2+i, j] = Wio_s[(2t+i)*128+p, mo*128+j]
    # whh: m-block-major [p, ((b*8+t)*2+mloc)*2+i, j]
    #        = Whh_s[(2t+i)*128+p, (2b+mloc)*128+j]
    # who: [p, (t*4+mo)*2+i, j] = Who_s[(2t+i)*128+p, mo*128+j]
    xT = nc.dram_tensor("xT", [128, NCH * 4, CH], FP8,
                        kind="ExternalInput").ap()
    wih = nc.dram_tensor("wih", [128, 8 * 4 * 2, 128], FP8,
                         kind="ExternalInput").ap()
    whh = nc.dram_tensor("whh", [128, KP * KH * 2, 128], FP8,
                         kind="ExternalInput").ap()
    who = nc.dram_tensor("who", [128, KP * KO * 2, 128], FP8,
                         kind="ExternalInput").ap()
    wio = nc.dram_tensor("wio", [128, 2 * KO * 2, 128], FP8,
                         kind="ExternalInput").ap()
    hbc = nc.dram_tensor("hbc", [128, KH], F32, kind="ExternalInput").ap()
    obc = nc.dram_tensor("obc", [128, KO], F32, kind="ExternalInput").ap()
    bmask = nc.dram_tensor("bmask", [128, 2 * CH], mybir.dt.uint8,
                           kind="ExternalInput").ap()
    outT = nc.dram_tensor("outT", [NO, BL], BF16, kind="ExternalOutput").ap()

    with tile.TileContext(nc) as tc:
        with tc.tile_pool(name="w", bufs=1) as wpool, \
             tc.tile_pool(name="act", bufs=1) as apool, \
             tc.tile_pool(name="ps", bufs=2, space="PSUM") as pspool, \
             tc.tile_pool(name="out", bufs=4) as opool:

            # ---- stage inputs. wih lands as one 128KB DMA per 2-m-tile
            # block (sync queue) in the exact order the P phase consumes
            # them; x lands chunk-major on the scalar queue; the 4MB whh
            # follows split across both queues.
            wih_m = wpool.tile([128, 8 * 4 * 2, 128], FP8, tag="wih",
                               name="wihm")
            x_m = wpool.tile([128, NCH * 4, CH], FP8, tag="x", name="xm")
            nc.sync.dma_start(wih_m[:, 0:4, :], wih[:, 0:4, :])
            nc.scalar.dma_start(x_m[:, 0:2, :], xT[:, 0:2, :])
            nc.scalar.dma_start(x_m[:, 4:6, :], xT[:, 4:6, :])
            nc.sync.dma_start(wih_m[:, 4:8, :], wih[:, 4:8, :])
            nc.scalar.dma_start(x_m[:, 2:4, :], xT[:, 2:4, :])
            nc.scalar.dma_start(x_m[:, 6:8, :], xT[:, 6:8, :])
            for b in range(1, 8):
                nc.sync.dma_start(wih_m[:, b * 8:(b + 1) * 8, :],
                                  wih[:, b * 8:(b + 1) * 8, :])
            hbc_t = wpool.tile([128, KH], F32, tag="hbc")
            nc.scalar.dma_start(hbc_t[:], hbc[:])
            obc_t = wpool.tile([128, KO], F32, tag="obc")
            nc.scalar.dma_start(obc_t[:], obc[:])
            bmask_t = wpool.tile([128, 2 * CH], mybir.dt.uint8, tag="bmask")
            nc.scalar.dma_start(bmask_t[:], bmask[:])
            wio_m = wpool.tile([128, 2 * KO * 2, 128], FP8, tag="wio",
                               name="wiom")
            nc.scalar.dma_start(wio_m[:], wio[:])
            # whh fp8: 4MB total as 8x 0.5MB DMAs over sync+scalar queues,
            # landing in the order hh step 1 consumes its 2-m-tile blocks
            whh_m = wpool.tile([128, KP * KH * 2, 128], FP8, tag="whh",
                               name="whhm")
            for b in range(8):
                eng = nc.sync if b % 2 == 0 else nc.scalar
                eng.dma_start(whh_m[:, b * 32:(b + 1) * 32, :],
                              whh[:, b * 32:(b + 1) * 32, :])
            who_m = wpool.tile([128, KP * KO * 2, 128], FP8, tag="who",
                               name="whom")
            nc.scalar.dma_start(who_m[:], who[:])

            def x8_ap(t, c):
                lo = (c * 2 + t) * 2
                return x_m[:, lo:lo + 2, :]

            def wih8_ap(t, m):
                lo = ((m // 2 * 2 + t) * 2 + m % 2) * 2
                return wih_m[:, lo:lo + 2, :]

            def wio8_ap(t, mo):
                lo = (t * KO + mo) * 2
                return wio_m[:, lo:lo + 2, :]

            def psum2(i):
                # two 2-bank accumulators live at once (one per chunk, or
                # pipelined across 2-m-tile blocks); bufs=2 each fills PSUM
                return pspool.tile([128, 2 * CH], F32,
                                   tag=("psA" if i % 2 == 0 else "psB"),
                                   bufs=2, name="psb")

            # ---- x-projection P (= SCALE*(hr*(x@Wih.T)+hb), bf16) and
            # first-step activations: fp8 DoubleRow (K=512 = 2 pairs), both
            # chunks per weight load, 2-m-tile blocks ----
            P = {}
            A = {}
            for c in range(NCH):
                P[c] = apool.tile([128, KH * CH], BF16, tag=f"P{c}",
                                  name=f"P{c}")
                A[c] = apool.tile([128, KH, CH], FP8, tag=f"A{c}", bufs=2,
                                  name=f"A1c{c}")
            for blk in range(8):
                pss = [psum2(0), psum2(1)]
                for t in range(2):
                    for mloc in range(2):
                        m = 2 * blk + mloc
                        for c in range(NCH):
                            nc.tensor.matmul(
                                pss[c][:, mloc * CH:(mloc + 1) * CH],
                                wih8_ap(t, m), x8_ap(t, c),
                                start=(t == 0), stop=(t == 1), perf_mode=DR)
                for c in range(NCH):
                    for mloc in range(2):
                        m = 2 * blk + mloc
                        nc.vector.tensor_scalar_add(
                            P[c][:, m * CH:(m + 1) * CH],
                            pss[c][:, mloc * CH:(mloc + 1) * CH],
                            hbc_t[:, m:m + 1])
                    # A1 = act(P/SCALE) straight from SBUF — frees the PSUM
                    # slot as soon as the adds have read it
                    _emit_hidden_act2(
                        nc, P[c][:, 2 * blk * CH:(2 * blk + 2) * CH],
                        blk, A[c], opool, bmask_t)

            # ---- whh-independent output x-projection (fills the window
            # while the 4MB whh load is still in flight); holds SCALE*or*
            # (x@Wio.T) ----
            outx = {}
            for c in range(NCH):
                outx[c] = apool.tile([128, KO * CH], BF16, tag=f"outx{c}",
                                     name=f"outx{c}")
            for half in range(2):
                pss = [psum2(0), psum2(1)]
                for t in range(2):
                    for mloc in range(2):
                        mo = 2 * half + mloc
                        for c in range(NCH):
                            nc.tensor.matmul(
                                pss[c][:, mloc * CH:(mloc + 1) * CH],
                                wio8_ap(t, mo), x8_ap(t, c),
                                start=(t == 0), stop=(t == 1), perf_mode=DR)
                for c in range(NCH):
                    nc.vector.tensor_copy(
                        outx[c][:, 2 * half * CH:(2 * half + 2) * CH],
                        pss[c][:])

            # ---- recurrent steps 2..4: fp8 DoubleRow, both chunks computed
            # back-to-back per weight load (the dedupe pass removes the
            # second LDWEIGHTS) ----
            def hh_step_fused(s):
                a_new = [apool.tile([128, KH, CH], FP8, tag=f"A{c}", bufs=2,
                                    name=f"A{s + 2}c{c}") for c in range(NCH)]
                for blk in range(8):
                    pss = [psum2(0), psum2(1)]
                    for t in range(KP):
                        for mloc in range(2):
                            m = 2 * blk + mloc
                            w2 = ((blk * KP + t) * 2 + mloc) * 2
                            for c in range(NCH):
                                nc.tensor.matmul(
                                    pss[c][:, mloc * CH:(mloc + 1) * CH],
                                    whh_m[:, w2:w2 + 2, :],
                                    A[c][:, 2 * t:2 * t + 2, :],
                                    start=(t == 0), stop=(t == KP - 1),
                                    perf_mode=DR)
                    for c in range(NCH):
                        # pre = psum + P into an fp16 SBUF temp: a single
                        # PSUM read frees the bank, and the ACT engine reads
                        # 16-bit sources at full rate
                        tmp = opool.tile([128, 2 * CH], F16, tag="pre",
                                         bufs=4, name="pre")
                        nc.vector.tensor_add(
                            tmp[:], pss[c][:],
                            P[c][:, 2 * blk * CH:(2 * blk + 2) * CH])
                        _emit_hidden_act2(nc, tmp, blk, a_new[c], opool,
                                          bmask_t)
                for c in range(NCH):
                    A[c] = a_new[c]

            for s in range(N_STEPS - 1):
                hh_step_fused(s)

            # ---- output layer (fp8 DoubleRow, same weight reuse) ----
            for mo in range(KO):
                pss = [psum2(0), psum2(1)]
                for t in range(KP):
                    w2 = (t * KO + mo) * 2
                    for c in range(NCH):
                        nc.tensor.matmul(
                            pss[c][:, 0:CH],
                            who_m[:, w2:w2 + 2, :],
                            A[c][:, 2 * t:2 * t + 2, :],
                            start=(t == 0), stop=(t == KP - 1),
                            perf_mode=DR)
                # half-tile evictions keep the post-matmul drain short
                for c in range(NCH):
                    for h in range(2):
                        lo, hi = h * (CH // 2), (h + 1) * (CH // 2)
                        to = opool.tile([128, CH // 2], F16, tag="preo",
                                        bufs=4, name="preo")
                        nc.vector.tensor_add(
                            to[:], pss[c][:, lo:hi],
                            outx[c][:, mo * CH + lo:mo * CH + hi])
                        o = opool.tile([128, CH // 2], BF16, tag="o", bufs=4,
                                       name="o")
                        nc.scalar.activation(o[:], to[:], AF.Sigmoid,
                                             bias=obc_t[:, mo:mo + 1],
                                             scale=INV)
                        eng = nc.sync if (c + h) % 2 == 0 else nc.scalar
                        eng.dma_start(
                            outT[mo * 128:(mo + 1) * 128,
                                 c * CH + lo:c * CH + hi],
                            o[:])

    _dedupe_ldweights(nc)
    nc.compile()
    return nc


_NC_CACHE = None


def _get_nc():
    global _NC_CACHE
    if _NC_CACHE is None:
        _NC_CACHE = _build_nc()
    return _NC_CACHE


def _make_bmask():
    m = np.zeros((128, 2 * CH), np.uint8)
    m[:_B1 - (_B1 // 128) * 128, 0:CH] = 1          # tile 5: parts < 43 tanh
    m[_B2 - (_B2 // 128) * 128:, CH:2 * CH] = 1     # tile 10: parts >= 86 relu
    return m


def _pack_dr(w_s, kp, mt):
    """(kp*256, mt*128) k-major weights -> DoubleRow layout
    [128, kp*mt*2, 128]: [p, (t*mt+m)*2+i, j] = w_s[(2t+i)*128+p, m*128+j]
    """
    fp8 = ml_dtypes.float8_e4m3
    w4 = w_s.reshape(kp, 2, 128, mt, 128)            # t, i, p, m, j
    return np.ascontiguousarray(
        w4.transpose(2, 0, 3, 1, 4).reshape(128, kp * mt * 2, 128)).astype(fp8)


def _prep_in_maps(inputs):
    bf = ml_dtypes.bfloat16
    x = np.asarray(inputs["inputs"], np.float32)
    hr = np.asarray(inputs["hidden_responses"], np.float32)[PERM]
    hb = np.asarray(inputs["hidden_biases"], np.float32)[PERM]
    orr = np.asarray(inputs["output_responses"], np.float32)
    ob = np.asarray(inputs["output_biases"], np.float32)

    wih_s = SCALE * (hr[:, None] *
                     np.asarray(inputs["input_to_hidden"], np.float32)[PERM]).T
    whh_s = SCALE * (hr[:, None] *
                     np.asarray(inputs["hidden_to_hidden"],
                                np.float32)[PERM][:, PERM]).T
    who_s = SCALE * (orr[:, None] *
                     np.asarray(inputs["hidden_to_output"],
                                np.float32)[:, PERM]).T
    wio_s = SCALE * (orr[:, None] *
                     np.asarray(inputs["input_to_output"], np.float32)).T

    fp8 = ml_dtypes.float8_e4m3
    # wih m-block-major DR: [p, ((b*2+t)*2+mloc)*2+i, j]
    #   = wih_s[(2t+i)*128+p, (2b+mloc)*128+j]
    wih_p = np.ascontiguousarray(
        wih_s.reshape(2, 2, 128, 8, 2, 128).transpose(2, 3, 0, 4, 1, 5)
        .reshape(128, 64, 128)).astype(fp8)

    shared = {
        "wih": wih_p,
        "whh": np.ascontiguousarray(
            whh_s.reshape(KP, 2, 128, 8, 2, 128).transpose(2, 3, 0, 4, 1, 5)
            .reshape(128, KP * KH * 2, 128)).astype(fp8),
        "who": _pack_dr(who_s, KP, KO),
        "wio": _pack_dr(wio_s, 2, KO),
        "hbc": np.ascontiguousarray(SCALE * hb.reshape(KH, 128).T),
        "obc": np.ascontiguousarray(ob.reshape(KO, 128).T),
        "bmask": _make_bmask(),
    }
    in_maps = []
    for c in range(N_CORES):
        m = dict(shared)
        # x chunk-major DR: [p, (ch*2+t)*2+i, col]
        #   = x.T[(2t+i)*128+p, ch*CH+col]
        xc = np.ascontiguousarray(x[c * BL:(c + 1) * BL].T)     # (NI, BL)
        m["xT"] = np.ascontiguousarray(
            xc.reshape(2, 2, 128, NCH, CH).transpose(2, 3, 0, 1, 4)
            .reshape(128, NCH * 4, CH)).astype(fp8)
        in_maps.append(m)
    return in_maps


def _run(inputs, trace=False, tmpdir=None):
    nc = _get_nc()
    in_maps = _prep_in_maps(inputs)
    res = run_bass_kernel_spmd(nc, in_maps, core_ids=list(range(N_CORES)),
                               trace=trace, tmpdir=tmpdir)
    out = np.empty((B, NO), np.float32)
    for c in range(N_CORES):
        out[c * BL:(c + 1) * BL] = res.results[c]["outT"].T.astype(np.float32)
    return out, res


def kernel(**inputs) -> np.ndarray:
    out, _ = _run(inputs, trace=False)
    return out


if __name__ == "__main__":
    rng = np.random.default_rng(0)
    ins = {
        "inputs": rng.standard_normal((B, NI), dtype=np.float32),
        "input_to_hidden": rng.standard_normal((NH, NI), dtype=np.float32) * 0.02,
        "hidden_to_hidden": rng.standard_normal((NH, NH), dtype=np.float32) * 0.02,
        "output_to_hidden": rng.standard_normal((NH, NO), dtype=np.float32) * 0.02,
        "input_to_output": rng.standard_normal((NO, NI), dtype=np.float32) * 0.02,
        "hidden_to_output": rng.standard_normal((NO, NH), dtype=np.float32) * 0.02,
        "output_to_output": rng.standard_normal((NO, NO), dtype=np.float32) * 0.02,
        "hidden_responses": rng.standard_normal(NH, dtype=np.float32) * 0.1 + 1.0,
        "hidden_biases": rng.standard_normal(NH, dtype=np.float32) * 0.1,
        "output_responses": rng.standard_normal(NO, dtype=np.float32) * 0.1 + 1.0,
        "output_biases": rng.standard_normal(NO, dtype=np.float32) * 0.1,
    }
    out = kernel(**ins)
    print("kernel output", out.shape, out.dtype, out[:2, :4])


# revision 29
# speedup vs baseline: 1.0203x; 1.0160x over previous
"""Trainium2 Bass kernel for a 4-step differentiable recurrent net forward pass.

Reference computation (B=8192, NI=512, NH=2048, NO=512, 4 steps):
    activs = 0; outputs = 0
    repeat 4x:  pre = hr * (x @ Wih.T + activs @ Whh.T + outputs @ Woh.T) + hb
                activs = per_neuron_act(pre)        # tanh/sigmoid/relu by i%3
    out = sigmoid(or * (x @ Wio.T + outputs @ Woo.T + activs @ Who.T) + ob)

`outputs` is never written inside the loop, so the Woh/Woo terms vanish and
the x-projection P = hr*(x@Wih.T)+hb is loop-invariant (computed once).

Strategy: data-parallel on batch across 8 cores (1024 rows each). On-core
everything is feature-major (features on SBUF partitions, batch on the free
axis), so each matmul is W_tile.T @ X^T with stationary weights. The
recurrent Whh and the output Who matmuls (91% of tensor work) run in
fp8 e4m3 with perf_mode=DoubleRow: K=256 contraction per instruction at 2
MACs/PE-cycle. DoubleRow LDWEIGHTS (256 columns, no FWL) costs more than
the matmul itself, so both 512-row batch chunks are computed back-to-back
under one weight load: bass emits an InstLdweights per matmul, and a
post-build pass (_dedupe_ldweights) removes the redundant second load so
the non-self-loading second matmul reuses the array-resident weights.
Weights are scaled x32 before the fp8 cast (keeps them out of the
subnormal range); every PSUM eviction goes through the ACT engine with
scale=1/32 to compensate. Eviction temporaries are fp16 (the ACT engine
reads 2B/cycle/lane, so f32 sources run at half rate) and activations are
written as fp8 directly by the ACT engine. The input projections (x@Wih,
x@Wio) are fp8 DoubleRow as well — simulated end-to-end rel err 1.27e-2
vs the 2e-2 budget. Host-side prep: hidden neurons are permuted
so the three activation groups are contiguous, hr/or are folded into the
weight matrices, weights are packed so each loads as one large contiguous
DMA, and hb/ob are applied as per-partition bias APs.
"""

import os

import numpy as np
import ml_dtypes

import concourse.bass as bass
import concourse.tile as tile
from concourse import bacc, mybir
from concourse.bass_utils import run_bass_kernel_spmd

B, NI, NH, NO = 8192, 512, 2048, 512
N_STEPS = 4
N_CORES = 8
BL = B // N_CORES          # batch rows per core
CH = 512                   # batch chunk (max moving free dim)
NCH = BL // CH             # 2 chunks per core
KI = NI // 128             # 4 k-tiles over inputs
KH = NH // 128             # 16 k/m-tiles over hidden
KP = KH // 2               # 8 DoubleRow k-pairs over hidden
KO = NO // 128             # 4 m-tiles over outputs

BF16 = mybir.dt.bfloat16
F16 = mybir.dt.float16
F32 = mybir.dt.float32
FP8 = mybir.dt.float8e4
AF = mybir.ActivationFunctionType
DR = mybir.MatmulPerfMode.DoubleRow

SCALE = 32.0               # fp8 weight pre-scale; undone by ACT scale=1/SCALE
INV = 1.0 / SCALE

# hidden neurons regrouped as [all tanh | all sigmoid | all relu]
_idx = np.arange(NH)
PERM = np.concatenate([_idx[_idx % 3 == 0], _idx[_idx % 3 == 1], _idx[_idx % 3 == 2]])
_B1 = int((_idx % 3 == 0).sum())           # 683
_B2 = _B1 + int((_idx % 3 == 1).sum())     # 1366

# per m-tile: the single activation function, or None for the two mixed tiles
_TILE_FUNC = []
for _m in range(KH):
    _lo, _hi = _m * 128, (_m + 1) * 128
    _fs = set()
    for _f, _a, _b in ((AF.Tanh, 0, _B1), (AF.Sigmoid, _B1, _B2), (AF.Relu, _B2, NH)):
        if max(_lo, _a) < min(_hi, _b):
            _fs.add(_f)
    _TILE_FUNC.append(_fs.pop() if len(_fs) == 1 else None)

# mixed tiles: (major_func applied everywhere, minor_func, mask column block)
# partition sub-ranges must be 32-aligned on TRN2, so the minority strip is
# fixed up with a full-tile ACT + copy_predicated against a {0,1} mask
_BOUNDARY = {
    _B1 // 128: (AF.Sigmoid, AF.Tanh, 0),    # tile 5: parts < 43 are tanh
    _B2 // 128: (AF.Sigmoid, AF.Relu, 1),    # tile 10: parts >= 86 are relu
}


def _dedupe_ldweights(nc):
    """Drop an InstLdweights that reloads exactly what the PE already holds.

    bass splits every matmul into InstLdweights + non-self-loading
    InstMatmult at build time; consecutive matmuls on the same stationary
    tile therefore carry a redundant (and expensive, for DoubleRow) reload.
    Only sync-free duplicates are removed, and any non-matmul PE
    instruction invalidates the tracked weights.
    """
    removed = 0
    for blk in nc.main_func.blocks:
        prev_key = None
        to_remove = []
        for i in blk.instructions:
            tn = type(i).__name__
            if tn == "InstLdweights":
                k = (repr(i.ins[0]), repr(i.perf_mode), repr(i.is_transpose),
                     repr(i.tile_position), repr(i.tile_size))
                si = i.sync_info
                clean = si is None or (len(si.on_wait) == 0
                                       and len(si.on_update) == 0)
                if k == prev_key and clean:
                    to_remove.append(i)
                else:
                    prev_key = k
            elif tn == "InstMatmult":
                continue
            elif getattr(i, "engine", None) == mybir.EngineType.PE:
                prev_key = None
        for i in to_remove:
            blk.instructions.remove(i)
        removed += len(to_remove)
    return removed


def _emit_hidden_act2(nc, ps, blk2, a_new, tmp_pool, bmask_t):
    """Evict a 2-m-tile pre-activation slab through the grouped activations.

    ps:    SBUF AP (128, 2*CH) fp16/bf16 holding SCALE*pre for m-tiles
           2*blk2, 2*blk2+1
    a_new: SBUF tile (128, KH, CH) fp8, m-tile m lives at [:, m, :]
    """
    mloc = 0
    while mloc < 2:
        m = 2 * blk2 + mloc
        if m in _BOUNDARY:
            major, minor, mb = _BOUNDARY[m]
            nc.scalar.activation(
                a_new[:, m, :], ps[:, mloc * CH:(mloc + 1) * CH], major,
                scale=INV)
            t = tmp_pool.tile([128, CH], FP8, tag="btmp", bufs=2, name="btmp")
            nc.scalar.activation(t[:], ps[:, mloc * CH:(mloc + 1) * CH], minor,
                                 scale=INV)
            nc.vector.copy_predicated(
                a_new[:, m, :], bmask_t[:, mb * CH:(mb + 1) * CH], t[:])
            mloc += 1
            continue
        func = _TILE_FUNC[m]
        end = mloc + 1
        while end < 2 and _TILE_FUNC[2 * blk2 + end] == func:
            end += 1
        nc.scalar.activation(
            a_new[:, 2 * blk2 + mloc:2 * blk2 + end, :],
            ps[:, mloc * CH:end * CH], func, scale=INV)
        mloc = end


def _build_nc():
    nc = bacc.Bacc("TRN2", target_bir_lowering=False, debug=False,
                   num_devices=N_CORES, dynamic_dma_scratch_size=2048)

    # all operands fp8, host-packed for DoubleRow APs with DMA arrival order
    # matching compute order:
    # x:   [p, (c*2+t)*2+i, col] = x.T[(2t+i)*128+p, c*CH+col]
    # wih: m-block-major [p, ((b*2+t)*2+mloc)*2+i, j]
    #        = Wih_s[(2t+i)*128+p, (2b+mloc)*128+j]
    # wio: [p, (t*4+mo)*2+i, j] = Wio_s[(2t+i)*128+p, mo*128+j]
    # whh: m-block-major [p, ((b*8+t)*2+mloc)*2+i, j]
    #        = Whh_s[(2t+i)*128+p, (2b+mloc)*128+j]
    # who: [p, (t*4+mo)*2+i, j] = Who_s[(2t+i)*128+p, mo*128+j]
    xT = nc.dram_tensor("xT", [128, NCH * 4, CH], FP8,
                        kind="ExternalInput").ap()
    wih = nc.dram_tensor("wih", [128, 8 * 4 * 2, 128], FP8,
                         kind="ExternalInput").ap()
    whh = nc.dram_tensor("whh", [128, KP * KH * 2, 128], FP8,
                         kind="ExternalInput").ap()
    who = nc.dram_tensor("who", [128, KP * KO * 2, 128], FP8,
                         kind="ExternalInput").ap()
    wio = nc.dram_tensor("wio", [128, 2 * KO * 2, 128], FP8,
                         kind="ExternalInput").ap()
    hbc = nc.dram_tensor("hbc", [128, KH], F32, kind="ExternalInput").ap()
    obc = nc.dram_tensor("obc", [128, KO], F32, kind="ExternalInput").ap()
    bmask = nc.dram_tensor("bmask", [128, 2 * CH], mybir.dt.uint8,
                           kind="ExternalInput").ap()
    outT = nc.dram_tensor("outT", [NO, BL], BF16, kind="ExternalOutput").ap()

    with tile.TileContext(nc) as tc:
        with tc.tile_pool(name="w", bufs=1) as wpool, \
             tc.tile_pool(name="act", bufs=1) as apool, \
             tc.tile_pool(name="ps", bufs=2, space="PSUM") as pspool, \
             tc.tile_pool(name="out", bufs=4) as opool:

            # ---- stage inputs. wih lands as one 128KB DMA per 2-m-tile
            # block (sync queue) in the exact order the P phase consumes
            # them; x lands chunk-major on the scalar queue; the 4MB whh
            # follows split across both queues.
            wih_m = wpool.tile([128, 8 * 4 * 2, 128], FP8, tag="wih",
                               name="wihm")
            x_m = wpool.tile([128, NCH * 4, CH], FP8, tag="x", name="xm")
            nc.sync.dma_start(wih_m[:, 0:4, :], wih[:, 0:4, :])
            nc.scalar.dma_start(x_m[:, 0:2, :], xT[:, 0:2, :])
            nc.scalar.dma_start(x_m[:, 4:6, :], xT[:, 4:6, :])
            nc.sync.dma_start(wih_m[:, 4:8, :], wih[:, 4:8, :])
            nc.scalar.dma_start(x_m[:, 2:4, :], xT[:, 2:4, :])
            nc.scalar.dma_start(x_m[:, 6:8, :], xT[:, 6:8, :])
            for b in range(1, 8):
                nc.sync.dma_start(wih_m[:, b * 8:(b + 1) * 8, :],
                                  wih[:, b * 8:(b + 1) * 8, :])
            hbc_t = wpool.tile([128, KH], F32, tag="hbc")
            nc.scalar.dma_start(hbc_t[:], hbc[:])
            obc_t = wpool.tile([128, KO], F32, tag="obc")
            nc.scalar.dma_start(obc_t[:], obc[:])
            bmask_t = wpool.tile([128, 2 * CH], mybir.dt.uint8, tag="bmask")
            nc.scalar.dma_start(bmask_t[:], bmask[:])
            wio_m = wpool.tile([128, 2 * KO * 2, 128], FP8, tag="wio",
                               name="wiom")
            nc.scalar.dma_start(wio_m[:], wio[:])
            # whh fp8: 4MB total as 8x 0.5MB DMAs over sync+scalar queues,
            # landing in the order hh step 1 consumes its 2-m-tile blocks
            whh_m = wpool.tile([128, KP * KH * 2, 128], FP8, tag="whh",
                               name="whhm")
            for b in range(8):
                eng = nc.sync if b % 2 == 0 else nc.scalar
                eng.dma_start(whh_m[:, b * 32:(b + 1) * 32, :],
                              whh[:, b * 32:(b + 1) * 32, :])
            who_m = wpool.tile([128, KP * KO * 2, 128], FP8, tag="who",
                               name="whom")
            nc.scalar.dma_start(who_m[:], who[:])

            def x8_ap(t, c):
                lo = (c * 2 + t) * 2
                return x_m[:, lo:lo + 2, :]

            def wih8_ap(t, m):
                lo = ((m // 2 * 2 + t) * 2 + m % 2) * 2
                return wih_m[:, lo:lo + 2, :]

            def wio8_ap(t, mo):
                lo = (t * KO + mo) * 2
                return wio_m[:, lo:lo + 2, :]

            def psum2(i):
                # two 2-bank accumulators live at once (one per chunk, or
                # pipelined across 2-m-tile blocks); bufs=2 each fills PSUM
                return pspool.tile([128, 2 * CH], F32,
                                   tag=("psA" if i % 2 == 0 else "psB"),
                                   bufs=2, name="psb")

            # ---- x-projection P (= SCALE*(hr*(x@Wih.T)+hb), bf16) and
            # first-step activations: fp8 DoubleRow (K=512 = 2 pairs), both
            # chunks per weight load, 2-m-tile blocks ----
            P = {}
            A = {}
            for c in range(NCH):
                P[c] = apool.tile([128, KH * CH], BF16, tag=f"P{c}",
                                  name=f"P{c}")
                A[c] = apool.tile([128, KH, CH], FP8, tag=f"A{c}", bufs=2,
                                  name=f"A1c{c}")
            for blk in range(8):
                pss = [psum2(0), psum2(1)]
                for t in range(2):
                    for mloc in range(2):
                        m = 2 * blk + mloc
                        for c in range(NCH):
                            nc.tensor.matmul(
                                pss[c][:, mloc * CH:(mloc + 1) * CH],
                                wih8_ap(t, m), x8_ap(t, c),
                                start=(t == 0), stop=(t == 1), perf_mode=DR)
                hb_b = hbc_t[:, 2 * blk:2 * blk + 2].unsqueeze(2) \
                    .broadcast_to([128, 2, CH])
                for c in range(NCH):
                    # single broadcast add per chunk: P slab = psum + hb
                    nc.vector.tensor_add(
                        P[c][:, 2 * blk * CH:(2 * blk + 2) * CH],
                        pss[c][:], hb_b)
                    # A1 = act(P/SCALE) straight from SBUF — frees the PSUM
                    # slot as soon as the adds have read it
                    _emit_hidden_act2(
                        nc, P[c][:, 2 * blk * CH:(2 * blk + 2) * CH],
                        blk, A[c], opool, bmask_t)

            # ---- recurrent steps 2..4: fp8 DoubleRow, both chunks computed
            # back-to-back per weight load (the dedupe pass removes the
            # second LDWEIGHTS) ----
            def hh_step_fused(s):
                a_new = [apool.tile([128, KH, CH], FP8, tag=f"A{c}", bufs=2,
                                    name=f"A{s + 2}c{c}") for c in range(NCH)]
                for blk in range(8):
                    pss = [psum2(0), psum2(1)]
                    for t in range(KP):
                        for mloc in range(2):
                            m = 2 * blk + mloc
                            w2 = ((blk * KP + t) * 2 + mloc) * 2
                            for c in range(NCH):
                                nc.tensor.matmul(
                                    pss[c][:, mloc * CH:(mloc + 1) * CH],
                                    whh_m[:, w2:w2 + 2, :],
                                    A[c][:, 2 * t:2 * t + 2, :],
                                    start=(t == 0), stop=(t == KP - 1),
                                    perf_mode=DR)
                    for c in range(NCH):
                        # pre = psum + P into an fp16 SBUF temp: a single
                        # PSUM read frees the bank, and the ACT engine reads
                        # 16-bit sources at full rate
                        tmp = opool.tile([128, 2 * CH], F16, tag="pre",
                                         bufs=4, name="pre")
                        nc.vector.tensor_add(
                            tmp[:], pss[c][:],
                            P[c][:, 2 * blk * CH:(2 * blk + 2) * CH])
                        _emit_hidden_act2(nc, tmp, blk, a_new[c], opool,
                                          bmask_t)
                for c in range(NCH):
                    A[c] = a_new[c]

            for s in range(N_STEPS - 1):
                hh_step_fused(s)

            # ---- output x-projection (deferred to here so its PSUM
            # evictions don't contend with the DVE-bound P phase; the
            # eviction copies hide under the who matmuls); holds
            # SCALE*or*(x@Wio.T) ----
            outx = {}
            for c in range(NCH):
                outx[c] = apool.tile([128, KO * CH], BF16, tag=f"outx{c}",
                                     name=f"outx{c}")
            for half in range(2):
                pss = [psum2(0), psum2(1)]
                for t in range(2):
                    for mloc in range(2):
                        mo = 2 * half + mloc
                        for c in range(NCH):
                            nc.tensor.matmul(
                                pss[c][:, mloc * CH:(mloc + 1) * CH],
                                wio8_ap(t, mo), x8_ap(t, c),
                                start=(t == 0), stop=(t == 1), perf_mode=DR)
                for c in range(NCH):
                    nc.vector.tensor_copy(
                        outx[c][:, 2 * half * CH:(2 * half + 2) * CH],
                        pss[c][:])

            # ---- output layer (fp8 DoubleRow, same weight reuse) ----
            for mo in range(KO):
                pss = [psum2(0), psum2(1)]
                for t in range(KP):
                    w2 = (t * KO + mo) * 2
                    for c in range(NCH):
                        nc.tensor.matmul(
                            pss[c][:, 0:CH],
                            who_m[:, w2:w2 + 2, :],
                            A[c][:, 2 * t:2 * t + 2, :],
                            start=(t == 0), stop=(t == KP - 1),
                            perf_mode=DR)
                # half-tile evictions keep the post-matmul drain short
                for c in range(NCH):
                    for h in range(2):
                        lo, hi = h * (CH // 2), (h + 1) * (CH // 2)
                        to = opool.tile([128, CH // 2], F16, tag="preo",
                                        bufs=4, name="preo")
                        nc.vector.tensor_add(
                            to[:], pss[c][:, lo:hi],
                            outx[c][:, mo * CH + lo:mo * CH + hi])
                        o = opool.tile([128, CH // 2], BF16, tag="o", bufs=4,
                                       name="o")
                        nc.scalar.activation(o[:], to[:], AF.Sigmoid,
                                             bias=obc_t[:, mo:mo + 1],
                                             scale=INV)
                        eng = nc.sync if (c + h) % 2 == 0 else nc.scalar
                        eng.dma_start(
                            outT[mo * 128:(mo + 1) * 128,
                                 c * CH + lo:c * CH + hi],
                            o[:])

    _dedupe_ldweights(nc)
    nc.compile()
    return nc


_NC_CACHE = None


def _get_nc():
    global _NC_CACHE
    if _NC_CACHE is None:
        _NC_CACHE = _build_nc()
    return _NC_CACHE


def _make_bmask():
    m = np.zeros((128, 2 * CH), np.uint8)
    m[:_B1 - (_B1 // 128) * 128, 0:CH] = 1          # tile 5: parts < 43 tanh
    m[_B2 - (_B2 // 128) * 128:, CH:2 * CH] = 1     # tile 10: parts >= 86 relu
    return m


def _pack_dr(w_s, kp, mt):
    """(kp*256, mt*128) k-major weights -> DoubleRow layout
    [128, kp*mt*2, 128]: [p, (t*mt+m)*2+i, j] = w_s[(2t+i)*128+p, m*128+j]
    """
    fp8 = ml_dtypes.float8_e4m3
    w4 = w_s.reshape(kp, 2, 128, mt, 128)            # t, i, p, m, j
    return np.ascontiguousarray(
        w4.transpose(2, 0, 3, 1, 4).reshape(128, kp * mt * 2, 128)).astype(fp8)


def _prep_in_maps(inputs):
    bf = ml_dtypes.bfloat16
    x = np.asarray(inputs["inputs"], np.float32)
    hr = np.asarray(inputs["hidden_responses"], np.float32)[PERM]
    hb = np.asarray(inputs["hidden_biases"], np.float32)[PERM]
    orr = np.asarray(inputs["output_responses"], np.float32)
    ob = np.asarray(inputs["output_biases"], np.float32)

    wih_s = SCALE * (hr[:, None] *
                     np.asarray(inputs["input_to_hidden"], np.float32)[PERM]).T
    whh_s = SCALE * (hr[:, None] *
                     np.asarray(inputs["hidden_to_hidden"],
                                np.float32)[PERM][:, PERM]).T
    who_s = SCALE * (orr[:, None] *
                     np.asarray(inputs["hidden_to_output"],
                                np.float32)[:, PERM]).T
    wio_s = SCALE * (orr[:, None] *
                     np.asarray(inputs["input_to_output"], np.float32)).T

    fp8 = ml_dtypes.float8_e4m3
    # wih m-block-major DR: [p, ((b*2+t)*2+mloc)*2+i, j]
    #   = wih_s[(2t+i)*128+p, (2b+mloc)*128+j]
    wih_p = np.ascontiguousarray(
        wih_s.reshape(2, 2, 128, 8, 2, 128).transpose(2, 3, 0, 4, 1, 5)
        .reshape(128, 64, 128)).astype(fp8)

    shared = {
        "wih": wih_p,
        "whh": np.ascontiguousarray(
            whh_s.reshape(KP, 2, 128, 8, 2, 128).transpose(2, 3, 0, 4, 1, 5)
            .reshape(128, KP * KH * 2, 128)).astype(fp8),
        "who": _pack_dr(who_s, KP, KO),
        "wio": _pack_dr(wio_s, 2, KO),
        "hbc": np.ascontiguousarray(SCALE * hb.reshape(KH, 128).T),
        "obc": np.ascontiguousarray(ob.reshape(KO, 128).T),
        "bmask": _make_bmask(),
    }
    in_maps = []
    for c in range(N_CORES):
        m = dict(shared)
        # x chunk-major DR: [p, (ch*2+t)*2+i, col]
        #   = x.T[(2t+i)*128+p, ch*CH+col]
        xc = np.ascontiguousarray(x[c * BL:(c + 1) * BL].T)     # (NI, BL)
        m["xT"] = np.ascontiguousarray(
            xc.reshape(2, 2, 128, NCH, CH).transpose(2, 3, 0, 1, 4)
            .reshape(128, NCH * 4, CH)).astype(fp8)
        in_maps.append(m)
    return in_maps


def _run(inputs, trace=False, tmpdir=None):
    nc = _get_nc()
    in_maps = _prep_in_maps(inputs)
    res = run_bass_kernel_spmd(nc, in_maps, core_ids=list(range(N_CORES)),
                               trace=trace, tmpdir=tmpdir)
    out = np.empty((B, NO), np.float32)
    for c in range(N_CORES):
        out[c * BL:(c + 1) * BL] = res.results[c]["outT"].T.astype(np.float32)
    return out, res


def kernel(**inputs) -> np.ndarray:
    out, _ = _run(inputs, trace=False)
    return out


if __name__ == "__main__":
    rng = np.random.default_rng(0)
    ins = {
        "inputs": rng.standard_normal((B, NI), dtype=np.float32),
        "input_to_hidden": rng.standard_normal((NH, NI), dtype=np.float32) * 0.02,
        "hidden_to_hidden": rng.standard_normal((NH, NH), dtype=np.float32) * 0.02,
        "output_to_hidden": rng.standard_normal((NH, NO), dtype=np.float32) * 0.02,
        "input_to_output": rng.standard_normal((NO, NI), dtype=np.float32) * 0.02,
        "hidden_to_output": rng.standard_normal((NO, NH), dtype=np.float32) * 0.02,
        "output_to_output": rng.standard_normal((NO, NO), dtype=np.float32) * 0.02,
        "hidden_responses": rng.standard_normal(NH, dtype=np.float32) * 0.1 + 1.0,
        "hidden_biases": rng.standard_normal(NH, dtype=np.float32) * 0.1,
        "output_responses": rng.standard_normal(NO, dtype=np.float32) * 0.1 + 1.0,
        "output_biases": rng.standard_normal(NO, dtype=np.float32) * 0.1,
    }
    out = kernel(**ins)
    print("kernel output", out.shape, out.dtype, out[:2, :4])
